# revision 37
# baseline (speedup 1.0000x reference)
"""Causal MHA + RoPE on 8 TRN2 NeuronCores — v5.

Sharding: 2 batch groups x 4 head-groups (4 heads each, EL=256 channels).
Each core: QKV proj (its heads) + RoPE + causal attention + O-proj partial;
host sums the 4 head-group partials per batch.

v5 vs v4:
- Host pre-permutes x/W to partition-major DRAM layouts so every input DMA
  is a large contiguous-line transfer on a HW-DGE queue (scalar+sync).
- One persistent x_sb tile; 8 x-chunk DMAs on sync, weights on scalar.
- exp stays on scalar exclusively during attention; QK-raw copies remain
  scalar but land in b-phase windows where scalar is idle.
- RoPE swap copies split vector/gpsimd; mask muls split gpsimd/vector.
- normalize: reciprocal straight from PSUM (no lcp copy).
- emit_e: fused [128,1024] ysb (vector+gpsimd halves), single bf16 y DMA
  per 128-token tile on sync; host upcasts and sums.
- warm_ps removed (tile-validation min-join fallback).
"""
import math
import numpy as np
import ml_dtypes

import concourse.bass as bass
import concourse.mybir as mybir
import concourse.tile as tile
from concourse import bacc
from concourse.bass import ds
from concourse.bass_utils import run_bass_kernel_spmd

F32 = mybir.dt.float32
BF16 = mybir.dt.bfloat16
F8 = mybir.dt.float8e4
EXP = mybir.ActivationFunctionType.Exp
# Score shift for fp8e4 exp outputs; cancels in softmax. This float8e4 has
# max finite 240 (inf above), so with data max score 8.33 we need B >= 2.86;
# the min single-entry row score is -3.02, needing B <= 3.22 to stay above
# the 2^-9 subnormal floor.
EXPB = 3.0

D_MODEL = 1024
DK = 64
THETA = 10000.0
B, S = 2, 2048
HPC = 4
EL = HPC * DK
SCALE = 1.0 / math.sqrt(DK)
NQ = 512
NT = 128
NSPAN = S // NQ
NKT = S // NT
DCH = D_MODEL // 128
VW = DK + 1
VWP = 68  # padded V block: HPC*VWP stride is 16B-aligned for DoubleRow

_CACHE = {}


def _build_nc():
    nc = bacc.Bacc(None, target_bir_lowering=False)
    # all inputs partition-major: [128, ...] with contiguous per-partition lines
    xT = nc.declare_dram_parameter("xT", [128, DCH * S], BF16, isOutput=False)
    wq = nc.declare_dram_parameter("wq", [128, DCH * EL], BF16, isOutput=False)
    wk = nc.declare_dram_parameter("wk", [128, DCH * EL], BF16, isOutput=False)
    wv = nc.declare_dram_parameter("wv", [128, DCH * EL], BF16, isOutput=False)
    wo = nc.declare_dram_parameter("wo", [128, 2 * D_MODEL], BF16, isOutput=False)
    cosT = nc.declare_dram_parameter("cosT", [128, S], BF16, isOutput=False)
    sinT = nc.declare_dram_parameter("sinT", [128, S], BF16, isOutput=False)
    y = nc.declare_dram_parameter("y", [S, D_MODEL], BF16, isOutput=True)

    with tile.TileContext(nc) as tc:
        with (
            tc.tile_pool(name="p_fin", bufs=1) as p_fin,
            tc.tile_pool(name="p_work", bufs=1) as p_work,
            tc.tile_pool(name="ps", bufs=1, space="PSUM") as ps,
        ):
            # ---- persistent tiles ----
            x_sb = p_fin.tile([128, DCH * S], BF16, tag="x_sb", name="x_sb")
            wq_sb = p_fin.tile([128, DCH * EL], BF16, tag="wq", name="wq_sb")
            wk_sb = p_fin.tile([128, DCH * EL], BF16, tag="wk", name="wk_sb")
            wv_sb = p_fin.tile([128, DCH * EL], BF16, tag="wv", name="wv_sb")
            wo_sb = p_fin.tile([128, 2 * D_MODEL], BF16, tag="wo", name="wo_sb")
            cos_sb = p_fin.tile([128, S], BF16, tag="cos", name="cos_sb")
            sin_sb = p_fin.tile([128, S], BF16, tag="sin", name="sin_sb")
            qt_fin = p_fin.tile([128, 2 * S], BF16, tag="qt_fin", name="qt_fin")
            kt_fin = p_fin.tile([128, 2 * S], BF16, tag="kt_fin", name="kt_fin")
            v_aug = p_fin.tile([128, NKT * HPC * VWP], BF16, tag="v_aug", name="v_aug")
            ones4 = p_fin.tile([128, HPC], F32, tag="ones4", name="ones4")
            nc.vector.memset(ones4, 1.0)
            nbias = p_fin.tile([128, 1], F32, tag="nbias", name="nbias")
            nc.vector.memset(nbias, -EXPB)
            attnT = [p_fin.tile([128, S], BF16, tag=f"attnT{p}", name=f"attnT{p}")
                     for p in range(2)]

            # ---- input DMAs: weights on scalar (HW-DGE), x on sync (HW-DGE).
            # DMA completion is round-robin across in-flight transfers, so
            # first-needed data would otherwise land last. Enforce FIFO with
            # lookahead-2 per engine: after issuing unit k+1, a tiny
            # dependent read of unit k blocks the engine queue until unit k's
            # transfer completes before unit k+2 can issue.
            xscr = p_fin.tile([1, 256], BF16, tag="xscr", name="xscr")
            wscr = p_fin.tile([1, 64], BF16, tag="wscr", name="wscr")

            NXC = 8  # x chunks of 256 tokens
            xcw = S // NXC

            def x_chunk_dma(xc):
                nc.sync.dma_start(
                    out=x_sb.rearrange("p (d t) -> p d t", d=DCH)[
                        :, :, xc * xcw:(xc + 1) * xcw],
                    in_=xT.rearrange("p (d t) -> p d t", d=DCH)[
                        :, :, xc * xcw:(xc + 1) * xcw])

            def x_fence(xc):
                # tiny SBUF->SBUF dma reading chunk xc's landing zone
                nc.sync.dma_start(
                    out=xscr[0:1, ds(32 * (xc % 8), 8)],
                    in_=x_sb[0:1, ds(xc * xcw, 8)])

            w_units = [(wq_sb, wq), (wk_sb, wk), (cos_sb, cosT),
                       (sin_sb, sinT), (wv_sb, wv), (wo_sb, wo)]

            def w_dma(k):
                sb, dr = w_units[k]
                nc.scalar.dma_start(out=sb, in_=dr[:, :])

            def w_fence(k):
                nc.scalar.copy(wscr[0:1, ds(8 * k, 8)], w_units[k][0][0:1, 0:8])

            # interleaved issue, lookahead 2 per engine; scalar's queue is
            # additionally gated on x chunk 1 so cos/sin/wk don't steal
            # bandwidth from the critical first x chunks
            x_chunk_dma(0)
            w_dma(0)                      # wq
            x_chunk_dma(1)
            x_fence(0)
            nc.scalar.copy(wscr[0:1, 56:64], x_sb[0:1, ds(1 * xcw, 8)])
            w_dma(1)                      # wk
            x_chunk_dma(2)
            w_fence(0)
            x_fence(1)
            x_chunk_dma(3)
            w_dma(2)                      # cos
            x_fence(2)
            w_fence(1)
            x_chunk_dma(4)
            w_dma(3)                      # sin
            x_fence(3)
            w_fence(2)
            x_chunk_dma(5)
            w_dma(4)                      # wv
            x_fence(4)
            w_fence(3)
            x_chunk_dma(6)
            w_dma(5)                      # wo
            x_fence(5)
            x_chunk_dma(7)

            # causal 0/1 masks for the 4 diagonal offsets, same per head half
            mask4 = []
            for dd in range(4):
                m = p_fin.tile([128, 2 * NQ], BF16, tag=f"mask{dd}", name=f"mask{dd}")
                nc.vector.memset(m, 1.0)
                nc.gpsimd.affine_select(
                    out=m.rearrange("p (h q) -> p h q", h=2),
                    in_=m.rearrange("p (h q) -> p h q", h=2),
                    compare_op=mybir.AluOpType.is_ge,
                    fill=0.0, base=-NT * dd,
                    pattern=[[0, 2], [1, NQ]],
                    channel_multiplier=-1,
                )
                mask4.append(m)

            def xsl(j2, off, w):
                # x_sb slice helper: tokens [j2*1024+off, +w) of d-chunk given later
                return j2 * 1024 + off, w

            # ---- phase emitters ----
            def emit_qk(j2, kind, fine=False):
                base = j2 * 1024
                w_sb = wq_sb if kind == 0 else wk_sb
                for c in range(2):
                    pq = ps.tile([128, 1024], F32, tag="big", bufs=2, name="pq")
                    # fine=256-token spans let the very first group start as
                    # soon as x chunk 0 lands; elsewhere 512-token spans keep
                    # the matmul cadence stream-bound (N=256 is LDWEIGHTS-bound)
                    nsp = 4 if (fine and c == 0) else 2
                    sw = 1024 // nsp
                    for q4 in range(nsp):
                        for d in range(DCH):
                            nc.tensor.matmul(
                                pq[:, ds(q4 * sw, sw)],
                                w_sb[:, ds(d * EL + c * 128, 128)],
                                x_sb[:, ds(d * S + base + q4 * sw, sw)],
                                start=(d == 0), stop=(d == DCH - 1))
                    raw = p_work.tile([128, 1024], BF16, tag="raw", bufs=6, name="raw")
                    nc.scalar.copy(raw, pq)
                    swp = p_work.tile([128, 1024], BF16, tag="swp", bufs=6, name="swp")
                    for blk in range(2):
                        b0 = blk * 64
                        nc.vector.tensor_copy(swp[b0:b0 + 32, :], raw[b0 + 32:b0 + 64, :])
                        nc.vector.tensor_copy(swp[b0 + 32:b0 + 64, :], raw[b0:b0 + 32, :])
                    sl2 = ds(base, 1024)
                    fin = qt_fin if kind == 0 else kt_fin
                    nc.vector.tensor_mul(raw, raw, cos_sb[:, sl2])
                    nc.vector.tensor_mul(swp, swp, sin_sb[:, sl2])
                    nc.vector.tensor_add(fin[:, ds(c * S + base, 1024)], raw, swp)

            def emit_v(j2, shalf):
                base = j2 * 1024
                for sti in range(4 * shalf, 4 * shalf + 4):
                    t = 8 * j2 + sti
                    pv = ps.tile([128, 1024], F32, tag="big", bufs=2, name="pv")
                    for d in range(DCH):
                        nc.tensor.matmul(
                            pv[:, 0:EL],
                            x_sb[:, ds(d * S + base + sti * 128, 128)],
                            wv_sb[:, ds(d * EL, EL)],
                            start=(d == 0), stop=(d == DCH - 1))
                    vview = v_aug[:, ds(t * HPC * VWP, HPC * VWP)].rearrange(
                        "p (h a) -> p h a", a=VWP)
                    nc.scalar.copy(vview[:, :, 0:DK],
                                   pv[:, 0:EL].rearrange("p (h m) -> p h m", m=DK))
                    nc.vector.tensor_copy(vview[:, :, DK:VW],
                                          ones4.rearrange("p (h o) -> p h o", o=1))

            def emit_b(j2):
                emit_qk(j2, 0, fine=(j2 == 0))
                emit_qk(j2, 1)
                emit_v(j2, 0)
                emit_v(j2, 1)

            def emit_e_sti(sti):
                ysb = p_work.tile([128, 1024], BF16, tag="ysb", bufs=4, name="ysb")
                for e2 in range(2):
                    py = ps.tile([128, NQ], F32, tag="eacc", bufs=2, name="py")
                    for c in range(2):
                        nc.tensor.matmul(
                            py[:, 0:NQ],
                            attnT[c][:, ds(sti * 128, 128)],
                            wo_sb[:, ds(c * D_MODEL + e2 * NQ, NQ)],
                            start=(c == 0), stop=(c == 1))
                    nc.vector.tensor_copy(ysb[:, ds(e2 * NQ, NQ)], py[:, 0:NQ])
                nc.sync.dma_start(
                    out=y[sti * 128:(sti + 1) * 128, :],
                    in_=ysb)

            def emit_attn(p, j, ejobs=()):
                pva = ps.tile([128, NQ], F32, tag="pacc", bufs=2, name="pva")
                pvb = ps.tile([128, NQ], F32, tag="pacc", bufs=2, name="pvb")
                ntile = 4 * j + 4
                for t in range(ntile):
                    dd = max(0, t - 4 * j)          # diagonal offset 0..3
                    q0 = dd * NT                    # first live q-col in this tile
                    w = NQ - q0                     # live width per head half
                    stp = ps.tile([128, 2 * NQ], F32, tag="big", bufs=2, name="stp")
                    for hh in range(2):
                        nc.tensor.matmul(
                            stp[:, ds(hh * NQ + q0, w)],
                            kt_fin[hh * 64:(hh + 1) * 64, ds(p * S + t * NT, NT)],
                            qt_fin[hh * 64:(hh + 1) * 64, ds(p * S + j * NQ + q0, w)],
                            start=True, stop=True)
                    ste = p_work.tile([128, 2 * NQ], BF16, tag="ste", bufs=8, name="ste")
                    wf = 2 * NQ - q0   # one flat segment; dead middle cols unread
                    nc.scalar.activation(ste[:, ds(q0, wf)], stp[:, ds(q0, wf)],
                                         EXP, scale=SCALE)
                    if t >= 4 * j:
                        nc.vector.tensor_mul(ste[:, ds(q0, wf)], ste[:, ds(q0, wf)],
                                             mask4[dd][:, ds(q0, wf)])
                    for hh, pvx in ((0, pva), (1, pvb)):
                        nc.tensor.matmul(
                            pvx[0:VW, ds(q0, w)],
                            v_aug[:, ds(t * HPC * VWP + (2 * p + hh) * VWP, VW)],
                            ste[:, ds(hh * NQ + q0, w)],
                            start=(t == 0), stop=(t == ntile - 1))
                    # fill tensor slack in this scalar(exp)-paced loop with
                    # O-projection groups of already-finished spans
                    for at, sti in ejobs:
                        if at == t:
                            emit_e_sti(sti)
                lcp_a = p_work.tile([1, NQ], F32, tag="lcp_a", bufs=3, name="lcp_a")
                lcp_b = p_work.tile([1, NQ], F32, tag="lcp_b", bufs=3, name="lcp_b")
                nc.vector.tensor_copy(lcp_a, pva[64:65, :])
                nc.vector.tensor_copy(lcp_b, pvb[64:65, :])
                recl_a = p_work.tile([1, NQ], F32, tag="recl_a", bufs=3, name="recl_a")
                recl_b = p_work.tile([1, NQ], F32, tag="recl_b", bufs=3, name="recl_b")
                nc.vector.reciprocal_approx_fast(recl_a, lcp_a)
                nc.vector.reciprocal_approx_fast(recl_b, lcp_b)
                rb_a = p_work.tile([64, NQ], F32, tag="rb_a", bufs=3, name="rb_a")
                rb_b = p_work.tile([64, NQ], F32, tag="rb_b", bufs=3, name="rb_b")
                nc.gpsimd.partition_broadcast(rb_a, recl_a, channels=64)
                nc.gpsimd.partition_broadcast(rb_b, recl_b, channels=64)
                sl = ds(j * NQ, NQ)
                nc.vector.tensor_mul(attnT[p][0:64, sl], pva[0:64, :], rb_a)
                nc.vector.tensor_mul(attnT[p][64:128, sl], pvb[0:64, :], rb_b)

            def emit_e(j):
                for sti in range(4 * j, 4 * j + 4):
                    emit_e_sti(sti)

            # PE warmup while the first x/wq DMAs land: the PE p-state ramps
            # 0.65 -> 1.2 -> 2.4 GHz with ~3us of continuous busy, so keep it
            # streaming garbage until real work arrives.
            warm = ps.tile([128, NQ], F32, tag="eacc", bufs=2, name="warm")
            for wi in range(40):
                nc.tensor.matmul(warm[0:4, 0:4], ones4[:, 0:4], ones4[:, 0:4],
                                 start=(wi == 0), stop=(wi == 39))

            # attn(p,0/1) only needs j2=0 outputs (qt/kt cols < 1024, key
            # tiles t < 8, v_aug t < 8), so they interleave with b1's
            # emission; the late phases then aren't all scalar(exp)-bound.
            emit_b(0)
            emit_attn(0, 0)
            emit_b(1)
            emit_attn(1, 0)
            emit_attn(0, 1)
            emit_e(0)
            emit_attn(1, 1)
            emit_attn(0, 2)
            emit_e(1)
            emit_attn(1, 2)
            emit_attn(0, 3)
            emit_e(2)
            emit_attn(1, 3)
            emit_e(3)
    nc.finalize()
    return nc


def _host_prep(x, Wq, Wk, Wv, Wo):
    x = np.asarray(x, dtype=np.float32)
    Wq, Wk, Wv, Wo = (np.asarray(w, dtype=np.float32) for w in (Wq, Wk, Wv, Wo))
    bf = ml_dtypes.bfloat16

    p64 = np.concatenate([np.arange(0, DK, 2), np.arange(1, DK, 2)])
    freqs = 1.0 / THETA ** (np.arange(0, DK, 2, dtype=np.float64) / DK)
    ang = np.arange(S, dtype=np.float64)[None, :] * freqs[:, None]
    cos32 = np.cos(ang).astype(np.float32)
    sin32 = np.sin(ang).astype(np.float32)
    cosT = np.ascontiguousarray(np.tile(cos32, (4, 1))).astype(bf)
    sinT = np.ascontiguousarray(
        np.concatenate([-sin32, sin32, -sin32, sin32], axis=0)).astype(bf)

    def pmajor(a):
        # [DCH*128, W] -> [128, DCH*W] with line (p, d*W+e) = a[d*128+p, e]
        w = a.shape[1]
        return np.ascontiguousarray(
            a.reshape(DCH_ROWS(a), 128, w).transpose(1, 0, 2).reshape(128, -1))

    def DCH_ROWS(a):
        return a.shape[0] // 128

    xTs = []
    for b in range(B):
        xt = np.ascontiguousarray(x[b].T)          # [1024, 2048]
        xTs.append(pmajor(xt).astype(bf))          # [128, 8*2048]
    perm = np.concatenate([h * DK + p64 for h in range(HPC)])

    in_maps = []
    for core in range(8):
        bg, hg = core // 4, core % 4
        sl = slice(hg * EL, (hg + 1) * EL)
        wq_h = pmajor(np.ascontiguousarray(Wq[sl][perm].T)).astype(bf)
        wk_h = pmajor(np.ascontiguousarray(Wk[sl][perm].T)).astype(bf)
        wv_h = pmajor(np.ascontiguousarray(Wv[sl].T)).astype(bf)
        wo_h = pmajor(np.ascontiguousarray(Wo[:, sl].T)).astype(bf)
        in_maps.append({
            "xT": xTs[bg],
            "wq": wq_h,
            "wk": wk_h,
            "wv": wv_h,
            "wo": wo_h,
            "cosT": cosT,
            "sinT": sinT,
        })
    return in_maps


def kernel(x, Wq, Wk, Wv, Wo, _trace=False):
    if "nc" not in _CACHE:
        _CACHE["nc"] = _build_nc()
    nc = _CACHE["nc"]
    in_maps = _host_prep(x, Wq, Wk, Wv, Wo)
    res = run_bass_kernel_spmd(nc, in_maps, core_ids=list(range(8)), trace=_trace)
    _CACHE["last_result"] = res
    out = np.zeros((B, S, D_MODEL), dtype=np.float32)
    for core in range(8):
        out[core // 4] += res.results[core]["y"].astype(np.float32)
    return out
